# revision 10
# baseline (speedup 1.0000x reference)
"""Multi-head attention (B=2, N=2048, C=768, H=12, DH=64) on 8 Trainium2 cores.

Sharding: data-parallel on batch (cores 0-3 -> b=0, cores 4-7 -> b=1),
tensor-parallel on heads within each group (3 heads/core: Wq/Wk/Wv column
slices, Wp row slices).  Each core emits its partial projection output;
the host sums the 4 partials per batch and adds bp.

Per-core dataflow (feature-major, transpose-free, fp16 operands / fp32 psum):
  - PE warm-up spinner: ~24 dummy matmuls bridge the input-DMA window so the
    PE HAM clock gate reaches 8/8 (2.4 GHz) before the projections start
  - qT,kT [64, N] per head = W.T @ xT; head-2 q/k halves merged into one
    M=128 matmul (q2|k2 column-concat weights), split by partition range
  - v [N, 192] token-major, ones column per head pre-memset (denominator)
  - ST [kj, qi] score tiles [128,512] per (kj-tile, head); two K=64 matmuls
    run concurrently on disjoint PE row halves (heads 0+1 paired; head 2
    pairs even/odd kj via partition-duplicated k/q); heads01 units first,
    head2 units last so yTA normalizes early
  - exp split across ACT (table exp) and DVE (Schraudolph int16-bitcast
    approx: i16 = s*a + b, bits reinterpreted as fp16); S runs 2 units
    ahead of PV
  - yT_aug[65, qi] = [v_h | 1].T @ ET accumulated over kj; row 64 = denom
  - normalize: denom row -> SBUF (ACT copy), reciprocal_approx_fast (DVE),
    gpsimd partition_broadcast, tensor_tensor multiply into yT fp16
  - out[qi, C] partial = yT stationary @ Wp: K=128 (h0+h1) + K=64 (h2);
    last qi block is split into A (h0+h1) and B (h2, extra out rows summed
    on host) so the projection overlaps the head-2 attention units
"""

import math
import os

import numpy as np

import concourse.bacc as bacc
import concourse.bass as bass
import concourse.mybir as mybir
import concourse.tile as tile
from concourse import bass_utils

B, N, C, H, DH = 2, 2048, 768, 12, 64
NCORES = 8
CPG = 4                  # cores per batch group
HPC = H // CPG           # heads per core = 3
MYC = HPC * DH           # per-core feature width = 192
KC = C // 128            # contraction chunks = 6
NTT = N // 128           # token tiles = 16
QB = 512                 # qi block width
F32 = mybir.dt.float32
MMDT = mybir.dt.float16
I16 = mybir.dt.int16
AF = mybir.ActivationFunctionType
OP = mybir.AluOpType

EXP_SHIFT = -3.0         # exp(s + EXP_SHIFT); cancels between num and denom
A_SCH = 1024.0 / math.log(2.0)            # fp16 Schraudolph scale
C_OPT = -0.044                            # minimax offset
B_SCH = 1024.0 * (15.0 + C_OPT) + EXP_SHIFT * A_SCH

# engine assignment pattern for exp tiles: A=ACT exact, D=DVE approx
EXP_PAT = os.environ.get("K_EXP_PAT", "AD")
N_WARM = int(os.environ.get("K_WARM", "28"))


def _emit(nc, tc, pools, aps):
    xT, wq, wk, wqk2, wv, wp = (aps[k] for k in
                                ("xT", "wq", "wk", "wqk2", "wv", "wp"))
    bq, bk, bqk2, bv, out = (aps[k] for k in ("bq", "bk", "bqk2", "bv", "out"))
    persist = pools["persist"]
    et_pool = pools["et"]
    small = pools["small"]
    ps = pools["ps"]

    # ---- persistent SBUF tensors ----
    xT_sb = persist.tile([128, KC * N], MMDT, tag="xT_sb")
    wq_sb = persist.tile([128, KC * 128], MMDT, tag="wq_sb")
    wk_sb = persist.tile([128, KC * 128], MMDT, tag="wk_sb")
    wqk2_sb = persist.tile([128, KC * 128], MMDT, tag="wqk2_sb")
    wv_sb = persist.tile([128, KC * MYC], MMDT, tag="wv_sb")
    wpA = persist.tile([128, C], MMDT, tag="wpA")
    wpB = persist.tile([64, C], MMDT, tag="wpB")
    bq_sb = persist.tile([128, 1], F32, tag="bq_sb")
    bk_sb = persist.tile([128, 1], F32, tag="bk_sb")
    bqk2_sb = persist.tile([128, 1], F32, tag="bqk2_sb")
    bv_row = persist.tile([1, MYC], MMDT, tag="bv_row")
    ones = persist.tile([1, 128], MMDT, tag="ones")
    shift_col = persist.tile([128, 1], F32, tag="shift_col")
    warm_sb = persist.tile([128, QB], MMDT, tag="warm_sb")
    qTA = persist.tile([128, N], MMDT, tag="qTA")
    kTA = persist.tile([128, N], MMDT, tag="kTA")
    # head 2 k/q duplicated on both partition halves (kj even/odd packing)
    qTB = persist.tile([128, N], MMDT, tag="qTB")
    kTB = persist.tile([128, N], MMDT, tag="kTB")
    v_sb = persist.tile([128, NTT * HPC * 65], MMDT, tag="v_sb")
    yTA = persist.tile([128, N], MMDT, tag="yTA")
    yTB = persist.tile([64, N], MMDT, tag="yTB")

    # ---- memsets first so the PE warm-up can start immediately ----
    nc.vector.memset(warm_sb, 0.5)
    ones_row_f32 = persist.tile([1, 128], F32, tag="ones_row_f32")
    nc.vector.memset(ones_row_f32, 1.0)
    nc.vector.tensor_copy(out=ones, in_=ones_row_f32)
    nc.vector.memset(shift_col, EXP_SHIFT)
    # ones columns of v_sb (denominator trick): memset whole tile once
    nc.vector.memset(v_sb, 1.0)

    # ---- input DMAs: xT chunks on sync, weights on gpsimd, bias on scalar
    for kc in range(KC):
        nc.sync.dma_start(out=xT_sb[:, kc * N:(kc + 1) * N],
                          in_=xT[kc * 128:(kc + 1) * 128, :])

    def whole_w_dma(eng, dst_sb, src, m):
        dst = bass.AP(tensor=dst_sb.tensor, offset=dst_sb.offset,
                      ap=[list(dst_sb.ap[0])] + [[m, KC], [1, m]])
        srcap = bass.AP(tensor=src.tensor, offset=src.offset,
                        ap=[[m, 128], [128 * m, KC], [1, m]])
        eng.dma_start(out=dst, in_=srcap)

    whole_w_dma(nc.gpsimd, wq_sb, wq, 128)
    whole_w_dma(nc.gpsimd, wk_sb, wk, 128)
    whole_w_dma(nc.gpsimd, wqk2_sb, wqk2, 128)
    whole_w_dma(nc.gpsimd, wv_sb, wv, MYC)
    nc.gpsimd.dma_start(out=wpA, in_=wp[0:128, :])
    nc.gpsimd.dma_start(out=wpB, in_=wp[128:MYC, :])
    nc.scalar.dma_start(out=bq_sb, in_=bq)
    nc.scalar.dma_start(out=bk_sb, in_=bk)
    nc.scalar.dma_start(out=bqk2_sb, in_=bqk2)
    nc.scalar.dma_start(out=bv_row, in_=bv)

    # ---- PE warm-up spinner (HAM clock-gate release while DMAs stream) ----
    if N_WARM:
        wps = ps.tile([128, QB], F32, tag="st", name="warm_ps")
        for _ in range(N_WARM):
            nc.tensor.matmul(wps, warm_sb[:, 0:128], warm_sb,
                             start=True, stop=True)

    # ---- phase 1: q/k projections (writebacks split across ACT/DVE) ----
    def wb_op(engine, dst, psrc, bias_sb):
        if engine == "A":
            nc.scalar.activation(dst, psrc, AF.Identity, bias=bias_sb)
        else:
            nc.vector.tensor_scalar(out=dst, in0=psrc, scalar1=bias_sb,
                                    scalar2=None, op0=OP.add)

    def qk_group(wsb, bias_sb, writeback, gname):
        pss = [ps.tile([128, QB], F32, tag="st", name=f"pqk{gname}_{_i}")
               for _i in range(N // QB)]
        for kc in range(KC):  # kc outer: overlap the xT load
            for nt in range(N // QB):
                nc.tensor.matmul(
                    pss[nt],
                    wsb[:, kc * 128:(kc + 1) * 128],
                    xT_sb[:, kc * N + nt * QB: kc * N + nt * QB + QB],
                    start=(kc == 0), stop=(kc == KC - 1),
                )
        for nt in range(N // QB):
            for eng, dst, d0, dlen, p0 in writeback:
                wb_op(eng, dst[d0:d0 + dlen, nt * QB:(nt + 1) * QB],
                      pss[nt][p0:p0 + dlen, :], bias_sb[p0:p0 + dlen, :])

    qk_group(wq_sb, bq_sb, [("A", qTA, 0, 128, 0)], "q")
    qk_group(wk_sb, bk_sb, [("D", kTA, 0, 128, 0)], "k")

    # ---- phase 2: v projection (token-major), before qk2 so the
    # attention heads01 units can start right after qk2's matmuls ----
    for nt in range(NTT):
        pv = ps.tile([128, MYC], F32, tag="st", name=f"pv{nt}")
        for kc in range(KC):
            nc.tensor.matmul(
                pv,
                xT_sb[:, kc * N + nt * 128: kc * N + nt * 128 + 128],
                wv_sb[:, kc * MYC:(kc + 1) * MYC],
                start=(kc == 0), stop=False,
            )
        nc.tensor.matmul(pv, ones[0:1, 0:128], bv_row, start=False, stop=True)
        # strided copy into v_sb (3 heads, 65-stride, ones col untouched)
        dst = bass.AP(tensor=v_sb.tensor,
                      offset=v_sb.offset + nt * HPC * 65,
                      ap=[list(v_sb.ap[0])] + [[65, HPC], [1, 64]])
        src = bass.AP(tensor=pv.tensor, offset=pv.offset,
                      ap=[list(pv.ap[0])] + [[64, HPC], [1, 64]])
        if nt % 2:
            nc.vector.tensor_copy(out=dst, in_=src)
        else:
            nc.scalar.activation(dst, src, AF.Copy)

    qk_group(wqk2_sb, bqk2_sb,
             [("A", qTB, 0, 64, 0), ("D", kTB, 0, 64, 64)], "qk2")
    # duplicate head-2 k/q onto partitions 64..127 (cross-partition: DMA)
    nc.gpsimd.dma_start(out=qTB[64:128, :], in_=qTB[0:64, :])
    nc.gpsimd.dma_start(out=kTB[64:128, :], in_=kTB[0:64, :])

    # ---- phase 3: attention ----
    def vh_ap(kj, h):
        base = (kj * HPC + h) * 65
        return v_sb[:, base:base + 65]

    exp_ctr = [0]

    def emit_exp(st_tile, name):
        eng = EXP_PAT[exp_ctr[0] % len(EXP_PAT)]
        exp_ctr[0] += 1
        if eng == "A":
            et = et_pool.tile([128, QB], MMDT, tag="et", name=f"et{name}")
            nc.scalar.activation(et, st_tile, AF.Exp, bias=shift_col[:, :])
            return et
        et_i = et_pool.tile([128, QB], I16, tag="et", name=f"et{name}")
        nc.vector.tensor_scalar(out=et_i, in0=st_tile, scalar1=A_SCH,
                                scalar2=B_SCH, op0=OP.mult, op1=OP.add)
        return et_i.bitcast(MMDT)

    def normalize(yt, ydst, q0, name):
        den = small.tile([1, QB], F32, tag="den", name=f"den{name}")
        nc.scalar.activation(den, yt[64:65, :], AF.Copy)
        rec = small.tile([1, QB], F32, tag="rec", name=f"rec{name}")
        nc.vector.reciprocal_approx_fast(out=rec, in_=den)
        bc = small.tile([64, QB], F32, tag="bc", name=f"bc{name}")
        nc.gpsimd.partition_broadcast(bc, rec)
        nc.vector.tensor_tensor(out=ydst[:, q0:q0 + QB], in0=yt[0:64, :],
                                in1=bc, op=OP.mult)

    def proj_qt(qt, part="AB"):
        # projection for one 128-row qi tile; psum carved from st-pool slots
        for nb in range(2):
            po = ps.tile([128, QB], F32, tag="st", name=f"pj{part}{qt}_{nb}")
            pslice = po[:, 0:384]
            if "A" in part:
                nc.tensor.matmul(pslice, yTA[:, qt * 128:(qt + 1) * 128],
                                 wpA[:, nb * 384:(nb + 1) * 384],
                                 start=True, stop=(part == "A"))
            if "B" in part:
                nc.tensor.matmul(pslice, yTB[0:64, qt * 128:(qt + 1) * 128],
                                 wpB[0:64, nb * 384:(nb + 1) * 384],
                                 start=(part == "B"), stop=True)
            row0 = qt * 128 if part != "B" else N + (qt - 12) * 128
            ob = pools["ostage"].tile([128, 384], MMDT, tag="ob",
                                      name=f"ob{part}{qt}_{nb}")
            if nb == 0:
                nc.scalar.activation(ob, pslice, AF.Copy)
            else:
                nc.vector.tensor_copy(out=ob, in_=pslice)
            nc.sync.dma_start(
                out=out[row0:row0 + 128, nb * 384:(nb + 1) * 384], in_=ob)

    # per qq block: 16 heads01 units (kj = i) then 8 head2 units (even/odd
    # kj pairs).  Each unit = 2 S-tiles -> 2 exp tiles -> 2 PV matmuls; S
    # runs LOOKAHEAD units ahead of PV.  Fixed-slot events interleave.
    LOOKAHEAD = 2
    NU = 16 + 8

    prev_yt2 = None
    for qq in range(4):
        q0 = qq * QB
        yt0 = ps.tile([65, QB], F32, tag="yt", bufs=3, name=f"yt0_{qq}")
        yt1 = ps.tile([65, QB], F32, tag="yt", bufs=3, name=f"yt1_{qq}")
        yt2 = None
        ets = {}

        def emit_S(i):
            if i < 16:  # heads 0+1, kj = i, row-paired on the PE array
                kj = i
                sa = ps.tile([128, QB], F32, tag="st", name=f"sA{qq}_{i}")
                nc.tensor.matmul(sa, kTA[0:64, kj * 128:(kj + 1) * 128],
                                 qTA[0:64, q0:q0 + QB], start=True, stop=True)
                sb = ps.tile([128, QB], F32, tag="st", name=f"sB{qq}_{i}")
                nc.tensor.matmul(sb, kTA[64:128, kj * 128:(kj + 1) * 128],
                                 qTA[64:128, q0:q0 + QB], start=True,
                                 stop=True)
            else:       # head2: kj pair (2k, 2k+1) on PE row halves
                kp = i - 16
                kj0, kj1 = 2 * kp, 2 * kp + 1
                sa = ps.tile([128, QB], F32, tag="st", name=f"sA{qq}_{i}")
                nc.tensor.matmul(sa, kTB[0:64, kj0 * 128:(kj0 + 1) * 128],
                                 qTB[0:64, q0:q0 + QB], start=True, stop=True)
                sb = ps.tile([128, QB], F32, tag="st", name=f"sB{qq}_{i}")
                nc.tensor.matmul(sb, kTB[64:128, kj1 * 128:(kj1 + 1) * 128],
                                 qTB[64:128, q0:q0 + QB], start=True,
                                 stop=True)
            ets[i] = (emit_exp(sa, f"a{qq}_{i}"), emit_exp(sb, f"b{qq}_{i}"))

        def emit_PV(i):
            ea, eb = ets.pop(i)
            if i < 16:
                kj = i
                nc.tensor.matmul(yt0, vh_ap(kj, 0), ea,
                                 start=(kj == 0), stop=(kj == 15))
                nc.tensor.matmul(yt1, vh_ap(kj, 1), eb,
                                 start=(kj == 0), stop=(kj == 15))
            else:
                kp = i - 16
                kj0, kj1 = 2 * kp, 2 * kp + 1
                nc.tensor.matmul(yt2, vh_ap(kj0, 2), ea,
                                 start=(kp == 0), stop=False)
                nc.tensor.matmul(yt2, vh_ap(kj1, 2), eb,
                                 start=False, stop=(kp == 7))

        for i in range(NU + LOOKAHEAD):
            if i == 16:
                yt2 = ps.tile([65, QB], F32, tag="yt", bufs=3,
                              name=f"yt2_{qq}")
            if i < NU:
                emit_S(i)
            if i >= LOOKAHEAD:
                emit_PV(i - LOOKAHEAD)
            # fixed-slot events (placed past their data deps):
            if qq > 0 and i == 2:
                normalize(prev_yt2, yTB, q0 - QB, f"2_{qq}")
            if qq > 0 and i in (5, 8, 11, 14):
                proj_qt((qq - 1) * 4 + (i - 5) // 3)
            if i == 19:
                normalize(yt0, yTA[0:64, :], q0, f"0_{qq}")
            if i == 20:
                normalize(yt1, yTA[64:128, :], q0, f"1_{qq}")
            if qq == 3 and i in (22, 23, 24, 25):
                proj_qt(12 + (i - 22), part="A")
        prev_yt2 = yt2

    # tail: last block's head-2 normalize + B-part projection (host sums)
    normalize(prev_yt2, yTB, 3 * QB, "2_t")
    for qt in range(12, 16):
        proj_qt(qt, part="B")


def _build_program():
    nc = bacc.Bacc("TRN2", target_bir_lowering=False, debug=False,
                   num_devices=NCORES)
    aps = {
        "xT": nc.dram_tensor("xT", [C, N], MMDT, kind="ExternalInput").ap(),
        "wq": nc.dram_tensor("wq", [C, 128], MMDT, kind="ExternalInput").ap(),
        "wk": nc.dram_tensor("wk", [C, 128], MMDT, kind="ExternalInput").ap(),
        "wqk2": nc.dram_tensor("wqk2", [C, 128], MMDT,
                               kind="ExternalInput").ap(),
        "wv": nc.dram_tensor("wv", [C, MYC], MMDT, kind="ExternalInput").ap(),
        "wp": nc.dram_tensor("wp", [MYC, C], MMDT, kind="ExternalInput").ap(),
        "bq": nc.dram_tensor("bq", [128, 1], F32, kind="ExternalInput").ap(),
        "bk": nc.dram_tensor("bk", [128, 1], F32, kind="ExternalInput").ap(),
        "bqk2": nc.dram_tensor("bqk2", [128, 1], F32,
                               kind="ExternalInput").ap(),
        "bv": nc.dram_tensor("bv", [1, MYC], MMDT, kind="ExternalInput").ap(),
        "out": nc.dram_tensor("out", [N + QB, C], MMDT,
                              kind="ExternalOutput").ap(),
    }
    with tile.TileContext(nc) as tc:
        import contextlib
        with contextlib.ExitStack() as ctx:
            pools = {
                "persist": ctx.enter_context(tc.tile_pool(name="persist",
                                                          bufs=1)),
                "et": ctx.enter_context(tc.tile_pool(name="et", bufs=8)),
                "small": ctx.enter_context(tc.tile_pool(name="small", bufs=3)),
                "ostage": ctx.enter_context(tc.tile_pool(name="ostage",
                                                         bufs=4)),
                "ps": ctx.enter_context(
                    tc.tile_pool(name="ps", bufs=5, space="PSUM")),
            }
            _emit(nc, tc, pools, aps)
    nc.compile()
    return nc


_PROGRAM_CACHE = {}


def _get_program():
    if "nc" not in _PROGRAM_CACHE:
        _PROGRAM_CACHE["nc"] = _build_program()
    return _PROGRAM_CACHE["nc"]


def make_in_maps(x, Wq, bq, Wk, bk, Wv, bv, Wp, bp):
    scale = np.float32(1.0 / math.sqrt(DH))
    xTb = [np.ascontiguousarray(x[b].T) for b in range(B)]
    wire = mybir.dt.np(MMDT)
    in_maps = []
    for c in range(NCORES):
        b, hg = c // CPG, c % CPG
        cols = slice(hg * MYC, (hg + 1) * MYC)
        Wq_c = Wq[:, cols] * scale
        Wk_c = Wk[:, cols]
        in_maps.append({
            "xT": xTb[b].astype(wire),
            "wq": np.ascontiguousarray(Wq_c[:, 0:128]).astype(wire),
            "wk": np.ascontiguousarray(Wk_c[:, 0:128]).astype(wire),
            "wqk2": np.ascontiguousarray(
                np.hstack([Wq_c[:, 128:192], Wk_c[:, 128:192]])).astype(wire),
            "wv": np.ascontiguousarray(Wv[:, cols]).astype(wire),
            "wp": np.ascontiguousarray(Wp[cols, :]).astype(wire),
            "bq": (bq[cols][0:128] * scale).reshape(128, 1).astype(np.float32),
            "bk": bk[cols][0:128].reshape(128, 1).astype(np.float32),
            "bqk2": np.concatenate([bq[cols][128:192] * scale,
                                    bk[cols][128:192]]).reshape(128, 1)
                      .astype(np.float32),
            "bv": bv[cols].reshape(1, MYC).astype(wire),
        })
    return in_maps


def assemble(results, bp):
    out = np.empty((B, N, C), np.float32)
    for b in range(B):
        acc = results[b * CPG]["out"].astype(np.float64)
        for c in range(b * CPG + 1, (b + 1) * CPG):
            acc = acc + results[c]["out"]
        main = acc[0:N]
        main[N - QB:N] += acc[N:N + QB]   # B-part of the last qi block
        out[b] = (main + bp.astype(np.float64)).astype(np.float32)
    return out


def kernel(x, Wq, bq, Wk, bk, Wv, bv, Wp, bp, **extra_kwargs):
    x = np.asarray(x, np.float32)
    Wq = np.asarray(Wq, np.float32)
    Wk = np.asarray(Wk, np.float32)
    Wv = np.asarray(Wv, np.float32)
    Wp = np.asarray(Wp, np.float32)
    bq = np.asarray(bq, np.float32)
    bk = np.asarray(bk, np.float32)
    bv = np.asarray(bv, np.float32)
    bp = np.asarray(bp, np.float32)

    nc = _get_program()
    in_maps = make_in_maps(x, Wq, bq, Wk, bk, Wv, bv, Wp, bp)
    res = bass_utils.run_bass_kernel_spmd(nc, in_maps,
                                          core_ids=list(range(NCORES)))
    return assemble(res.results, bp)
